# revision 9
# baseline (speedup 1.0000x reference)
"""Location-aware attention Trainium2 kernel (8 NeuronCores, SPMD).

Shards the batch (B=16) across 8 cores, 2 batches per core.  All weights
are replicated.  Host code only does layout preparation (transposes,
bf16 casts, im2col window view of att_prev, length mask); every FLOP of
the module itself (both linears, the location conv, tanh, softmax,
context reduction) runs on the NeuronCores.

Math per batch b (matches reference):
    pre_enc[t,e] = sum_d value[t,d] W_enc[e,d]            (+ b_enc later)
    dec[e]       = sum_d query[d] W_dec[e,d]
    conv[c,t]    = sum_k W_conv[c,k] xpad[t+k]            (same padding)
    att_conv[t,e]= sum_c conv[c,t] W_att[e,c]
                 = sum_k G[k,e] xpad[t+k],  G = W_conv^T @ W_att^T  (rank-10 fold)
    z[e,t]       = pre_enc^T + att_conv^T                 (PSUM accumulation)
    th[e,t]      = tanh(z + dec[e] + b_enc[e])            (fused ACT bias)
    s[t]         = sum_e w_g[e] th[e,t]    (b_g dropped: softmax-invariant)
    a[t]         = mask[t] exp(2 s[t]) / sum_t' mask exp(2 s[t'])
    ctx[d]       = sum_t a[t] value[t,d]
"""

import numpy as np
import ml_dtypes

B, T, D = 16, 4096, 512
C, FW = 10, 100
KW = 2 * FW + 1           # 201
SHARP = 2.0
N_CORES = 8
BPC = B // N_CORES        # 2 batches per core
XW = T + 128              # im2col free width 4224
XPAD = T + 256            # padded att_prev length 4352

BF16 = ml_dtypes.bfloat16

_CACHE = {}


def _build_program():
    import concourse.bass as bass  # noqa: F401
    import concourse.tile as tile
    from concourse import bacc, mybir

    dt = mybir.dt
    f32 = dt.float32
    bf = dt.bfloat16
    AF = mybir.ActivationFunctionType

    nc = bacc.Bacc(
        "TRN2",
        target_bir_lowering=False,
        debug=False,
        enable_asserts=False,
        num_devices=N_CORES,
    )

    # ---- I/O ----
    value_n = nc.dram_tensor("value_n", [BPC, T, D], bf, kind="ExternalInput").ap()
    value_t = nc.dram_tensor("value_t", [BPC, D, T], bf, kind="ExternalInput").ap()
    x_im2col = nc.dram_tensor("x_im2col", [BPC, 128, XW], bf, kind="ExternalInput").ap()
    query_t = nc.dram_tensor("query_t", [D, BPC], bf, kind="ExternalInput").ap()
    mask01 = nc.dram_tensor("mask01", [128, BPC * 32], f32, kind="ExternalInput").ap()
    w_enc_t = nc.dram_tensor("w_enc_t", [D, D], bf, kind="ExternalInput").ap()
    w_dec_t = nc.dram_tensor("w_dec_t", [D, D], bf, kind="ExternalInput").ap()
    w_att_t = nc.dram_tensor("w_att_t", [C, D], bf, kind="ExternalInput").ap()
    w_conv = nc.dram_tensor("w_conv", [C, KW], bf, kind="ExternalInput").ap()
    wg = nc.dram_tensor("wg", [D, 1], bf, kind="ExternalInput").ap()
    becdup = nc.dram_tensor("becdup", [128, 8], f32, kind="ExternalInput").ap()
    ones_row = nc.dram_tensor("ones_row", [1, 128], f32, kind="ExternalInput").ap()
    ones_col = nc.dram_tensor("ones_col", [128, 1], f32, kind="ExternalInput").ap()
    id1 = nc.dram_tensor("id1", [1, 1], f32, kind="ExternalInput").ap()
    id128 = nc.dram_tensor("id128", [128, 128], f32, kind="ExternalInput").ap()

    ctx_out = nc.dram_tensor("context", [BPC, D], f32, kind="ExternalOutput").ap()
    aw_out = nc.dram_tensor("att_weights", [BPC, T], f32, kind="ExternalOutput").ap()

    NG = T // 1024            # 4 groups of 1024 (z psum tiles)
    NE = D // 128             # 4 e-chunks
    NK = D // 128             # 4 d-chunks
    NC_ = T // 128            # 32 t-chunks of 128

    with tile.TileContext(nc) as tc:
        with (
            tc.tile_pool(name="consts", bufs=1) as cpool,
            tc.tile_pool(name="vt", bufs=2) as vtpool,
            tc.tile_pool(name="vn", bufs=2) as vnpool,
            tc.tile_pool(name="xp", bufs=2) as xpool,
            tc.tile_pool(name="tanh", bufs=6) as thpool,
            tc.tile_pool(name="small", bufs=2) as spool,
            tc.tile_pool(name="exps", bufs=3) as epool,
            tc.tile_pool(name="zps", bufs=2, space="PSUM") as zpool,
            tc.tile_pool(name="scps", bufs=2, space="PSUM") as scpool,
            tc.tile_pool(name="miscps", bufs=2, space="PSUM") as mpool,
        ):
            # ---- constant / weight loads ----
            we_sb = cpool.tile([128, NK, D], bf, tag="we")
            nc.sync.dma_start(we_sb[:], w_enc_t.rearrange("(k p) e -> p k e", p=128))
            wd_sb = cpool.tile([128, NK, D], bf, tag="wd")
            nc.sync.dma_start(wd_sb[:], w_dec_t.rearrange("(k p) e -> p k e", p=128))
            wa_sb = cpool.tile([C, D], bf, tag="wa")
            nc.sync.dma_start(wa_sb[:], w_att_t)
            wc_sb = cpool.tile([C, KW], bf, tag="wc")
            nc.sync.dma_start(wc_sb[:], w_conv)
            wg_sb = cpool.tile([128, NE], bf, tag="wg")
            nc.sync.dma_start(wg_sb[:], wg.rearrange("(j p) o -> p (j o)", p=128))
            q_sb = cpool.tile([128, NK, BPC], bf, tag="q")
            nc.sync.dma_start(q_sb[:], query_t.rearrange("(k p) b -> p k b", p=128))
            m01_sb = cpool.tile([128, BPC * 32], f32, tag="m01")
            nc.sync.dma_start(m01_sb[:], mask01)
            bec_sb = cpool.tile([128, 8], f32, tag="bec")
            nc.sync.dma_start(bec_sb[:], becdup)
            onesr_sb = cpool.tile([1, 128], f32, tag="onesr")
            nc.sync.dma_start(onesr_sb[:], ones_row)
            onesc_sb = cpool.tile([128, 1], f32, tag="onesc")
            nc.sync.dma_start(onesc_sb[:], ones_col)
            id1_sb = cpool.tile([1, 1], f32, tag="id1")
            nc.sync.dma_start(id1_sb[:], id1)
            id128_sb = cpool.tile([128, 128], f32, tag="id128")
            nc.sync.dma_start(id128_sb[:], id128)

            # ---- G = (conv ∘ att) folded weights:  G^T[k,e] = sum_c W_conv[c,k] W_att^T[c,e]
            g1_ps = zpool.tile([128, D], f32, tag="z")
            nc.tensor.matmul(g1_ps[:], wc_sb[:, 0:128], wa_sb[:])
            g1_sb = cpool.tile([128, D], bf, tag="g1")
            nc.vector.tensor_copy(g1_sb[:], g1_ps[:])
            g2_ps = zpool.tile([KW - 128, D], f32, tag="z")
            nc.tensor.matmul(g2_ps[:], wc_sb[:, 128:KW], wa_sb[:])
            g2_sb = cpool.tile([KW - 128, D], bf, tag="g2")
            nc.vector.tensor_copy(g2_sb[:], g2_ps[:])

            # ---- dec = W_dec @ query  -> bias[e, (j,b)] = dec + b_enc
            dec_ps = zpool.tile([128, NE * BPC], f32, tag="z")
            for j in range(NE):
                for k in range(NK):
                    nc.tensor.matmul(
                        dec_ps[:, BPC * j : BPC * (j + 1)],
                        wd_sb[:, k, 128 * j : 128 * (j + 1)],
                        q_sb[:, k, :],
                        start=(k == 0),
                        stop=(k == NK - 1),
                    )
            bias_sb = cpool.tile([128, NE * BPC], f32, tag="bias")
            nc.vector.tensor_add(bias_sb[:], dec_ps[:], bec_sb[:])

            for b in range(BPC):
                # ---- per-batch loads ----
                vt_sb = vtpool.tile([128, NK, T], bf, tag="vt")
                vt_src = value_t[b].rearrange("(k p) t -> p k t", p=128)
                for k in range(NK):
                    nc.sync.dma_start(vt_sb[:, k, :], vt_src[:, k, :])
                vn_sb = vnpool.tile([128, NC_, D], bf, tag="vn")
                vn_src = value_n[b].rearrange("(c p) d -> p c d", p=128)
                for u in range(4):
                    nc.sync.dma_start(
                        vn_sb[:, 8 * u : 8 * (u + 1), :], vn_src[:, 8 * u : 8 * (u + 1), :]
                    )
                x_sb = xpool.tile([128, XW], bf, tag="x")
                nc.sync.dma_start(x_sb[:], x_im2col[b])

                acol_ps = mpool.tile([128, NC_], f32, tag="m")

                # ---- z / tanh / scores / exp / transpose ----
                for g in range(NG):
                    th_tiles = []
                    for j in range(NE):
                        z_ps = zpool.tile([128, 1024], f32, tag="z")
                        for h in range(2):
                            t0 = 1024 * g + 512 * h
                            sl = slice(512 * h, 512 * (h + 1))
                            for k in range(NK):
                                nc.tensor.matmul(
                                    z_ps[:, sl],
                                    we_sb[:, k, 128 * j : 128 * (j + 1)],
                                    vt_sb[:, k, t0 : t0 + 512],
                                    start=(k == 0),
                                    stop=False,
                                )
                            nc.tensor.matmul(
                                z_ps[:, sl],
                                g1_sb[:, 128 * j : 128 * (j + 1)],
                                x_sb[:, t0 : t0 + 512],
                                start=False,
                                stop=False,
                            )
                            nc.tensor.matmul(
                                z_ps[:, sl],
                                g2_sb[:, 128 * j : 128 * (j + 1)],
                                x_sb[0 : KW - 128, t0 + 128 : t0 + 640],
                                start=False,
                                stop=True,
                            )
                        th = thpool.tile([128, 1024], bf, tag="th")
                        nc.scalar.activation(
                            th[:], z_ps[:], AF.Tanh,
                            bias=bias_sb[:, BPC * j + b : BPC * j + b + 1],
                        )
                        th_tiles.append(th)
                    for h in range(2):
                        t0 = 1024 * g + 512 * h
                        it = 2 * g + h
                        sc_ps = scpool.tile([1, 512], f32, tag="sc")
                        for j in range(NE):
                            nc.tensor.matmul(
                                sc_ps[:],
                                wg_sb[:, j : j + 1],
                                th_tiles[j][:, 512 * h : 512 * (h + 1)],
                                start=(j == 0),
                                stop=(j == NE - 1),
                            )
                        ex = epool.tile([1, 512], f32, tag="ex")
                        nc.scalar.activation(ex[:], sc_ps[:], AF.Exp, scale=SHARP)
                        for v in range(4):
                            nc.tensor.transpose(
                                acol_ps[:, 4 * it + v : 4 * it + v + 1],
                                ex[0:1, 128 * v : 128 * (v + 1)],
                                id1_sb[:],
                            )

                # ---- masked softmax normalization (column layout) ----
                acol_m = spool.tile([128, NC_], f32, tag="acolm")
                nc.vector.tensor_mul(
                    acol_m[:], acol_ps[:], m01_sb[:, 32 * b : 32 * (b + 1)]
                )
                colsum = spool.tile([128, 1], f32, tag="colsum")
                nc.vector.tensor_reduce(
                    colsum[:], acol_m[:], axis=mybir.AxisListType.X,
                    op=mybir.AluOpType.add,
                )
                ssum_ps = mpool.tile([1, 1], f32, tag="m")
                nc.tensor.matmul(ssum_ps[:], onesc_sb[:], colsum[:])
                ssum = spool.tile([1, 1], f32, tag="ssum")
                nc.vector.tensor_copy(ssum[:], ssum_ps[:])
                inv = spool.tile([1, 1], f32, tag="inv")
                nc.vector.reciprocal(inv[:], ssum[:])
                invb_ps = mpool.tile([128, 1], f32, tag="m")
                nc.tensor.matmul(invb_ps[:], onesr_sb[:], inv[:])
                invb_sb = spool.tile([128, 1], f32, tag="invb")
                nc.vector.tensor_copy(invb_sb[:], invb_ps[:])

                acol_bf = spool.tile([128, NC_], bf, tag="acolbf")
                nc.vector.tensor_scalar_mul(acol_bf[:], acol_m[:], invb_sb[:])
                acol_f = spool.tile([128, NC_], f32, tag="acolf")
                nc.vector.tensor_scalar_mul(acol_f[:], acol_m[:], invb_sb[:])

                # att_weights row output: transpose [128,32] -> [32,128]
                awt_ps = mpool.tile([NC_, 128], f32, tag="m")
                nc.tensor.transpose(awt_ps[:], acol_f[:], id128_sb[:])
                aw_sb = spool.tile([NC_, 128], f32, tag="awsb")
                nc.vector.tensor_copy(aw_sb[:], awt_ps[:])
                nc.sync.dma_start(
                    aw_out[b].rearrange("(c p) -> c p", p=128), aw_sb[:]
                )

                # ---- context ----
                ctx_ps = mpool.tile([1, D], f32, tag="m")
                for c in range(NC_):
                    nc.tensor.matmul(
                        ctx_ps[:],
                        acol_bf[:, c : c + 1],
                        vn_sb[:, c, :],
                        start=(c == 0),
                        stop=(c == NC_ - 1),
                    )
                ctx_sb = spool.tile([1, D], f32, tag="ctx")
                nc.vector.tensor_copy(ctx_sb[:], ctx_ps[:])
                nc.sync.dma_start(ctx_out[b : b + 1, :], ctx_sb[:])

    nc.compile()
    return nc


def _get_program():
    if "nc" not in _CACHE:
        _CACHE["nc"] = _build_program()
    return _CACHE["nc"]


def _host_prep(value, query, input_lengths, att_prev,
               W_enc, b_enc, W_dec, W_att, W_conv, w_g, b_g):
    """Build per-core input maps (layout prep only)."""
    value = np.asarray(value, np.float32)
    query = np.asarray(query, np.float32)
    lens = np.asarray(input_lengths).astype(np.int64)
    att_prev = np.asarray(att_prev, np.float32)

    vb = value.astype(BF16)                                   # [B,T,D]
    vtb = np.ascontiguousarray(vb.transpose(0, 2, 1))         # [B,D,T]
    qtb = np.ascontiguousarray(query.T).astype(BF16)          # [D,B]

    xpad = np.zeros((B, XPAD), np.float32)
    xpad[:, FW : FW + T] = att_prev
    xpad = xpad.astype(BF16)
    # X[b, r, u] = xpad[b, u + r]
    xw = np.lib.stride_tricks.sliding_window_view(xpad, XW, axis=1)
    xw = np.ascontiguousarray(xw[:, 0:128, :])                # [B,128,XW]

    # mask01[b][p, c] = 1 if t = 128 c + p < len_b else 0
    t_idx = np.arange(T).reshape(32, 128).T                   # [128(p), 32(c)]
    m01 = (t_idx[None, :, :] < lens[:, None, None]).astype(np.float32)  # [B,128,32]

    w_enc_t = np.ascontiguousarray(np.asarray(W_enc, np.float32).T).astype(BF16)
    w_dec_t = np.ascontiguousarray(np.asarray(W_dec, np.float32).T).astype(BF16)
    w_att_t = np.ascontiguousarray(np.asarray(W_att, np.float32).T).astype(BF16)
    w_conv_n = np.ascontiguousarray(np.asarray(W_conv, np.float32)[:, 0, :]).astype(BF16)
    wg_col = np.ascontiguousarray(np.asarray(w_g, np.float32).reshape(D, 1)).astype(BF16)

    be = np.asarray(b_enc, np.float32)
    becdup = np.zeros((128, 8), np.float32)
    for j in range(4):
        for b in range(BPC):
            becdup[:, BPC * j + b] = be[128 * j : 128 * (j + 1)]

    ones_row = np.ones((1, 128), np.float32)
    ones_col = np.ones((128, 1), np.float32)
    id1 = np.ones((1, 1), np.float32)
    id128 = np.eye(128, dtype=np.float32)

    in_maps = []
    for i in range(N_CORES):
        b0 = BPC * i
        sl = slice(b0, b0 + BPC)
        m01c = np.concatenate([m01[b0 + b] for b in range(BPC)], axis=1)  # [128, BPC*32]
        in_maps.append({
            "value_n": vb[sl],
            "value_t": vtb[sl],
            "x_im2col": xw[sl],
            "query_t": np.ascontiguousarray(qtb[:, sl]),
            "mask01": np.ascontiguousarray(m01c),
            "w_enc_t": w_enc_t,
            "w_dec_t": w_dec_t,
            "w_att_t": w_att_t,
            "w_conv": w_conv_n,
            "wg": wg_col,
            "becdup": becdup,
            "ones_row": ones_row,
            "ones_col": ones_col,
            "id1": id1,
            "id128": id128,
        })
    return in_maps


def run(trace=False, trace_kwargs=None, **inputs):
    from concourse.bass_utils import run_bass_kernel_spmd

    in_maps = _host_prep(**inputs)
    nc = _get_program()
    res = run_bass_kernel_spmd(
        nc, in_maps, core_ids=list(range(N_CORES)), trace=trace,
        **(trace_kwargs or {}),
    )
    ctx = np.concatenate([res.results[i]["context"] for i in range(N_CORES)], axis=0)
    aw = np.concatenate([res.results[i]["att_weights"] for i in range(N_CORES)], axis=0)
    return (ctx, aw), res


def kernel(**inputs):
    (ctx, aw), _ = run(trace=False, **inputs)
    return ctx, aw


if __name__ == "__main__":
    import reference

    inputs = {k: np.asarray(v) for k, v in reference.setup_inputs().items()}
    ctx, aw = kernel(**inputs)
    print("context", ctx.shape, "att_weights", aw.shape)


# revision 38
# speedup vs baseline: 1.1947x; 1.1947x over previous
"""Location-aware attention Trainium2 kernel (8 NeuronCores, SPMD).

Shards the batch (B=16) across 8 cores, 2 batches per core.  All weights
are replicated.  Host code only does layout preparation (transposes,
bf16 casts, im2col window view of att_prev, length mask); every FLOP of
the module itself (both linears, the location conv, tanh, softmax,
context reduction) runs on the NeuronCores.

Math per batch b (matches reference):
    pre_enc[t,e] = sum_d value[t,d] W_enc[e,d]            (+ b_enc later)
    dec[e]       = sum_d query[d] W_dec[e,d]
    att_conv[t,e]= sum_k G[k,e] xpad[t+k],  G = W_conv^T @ W_att^T  (rank-10 fold)
    z[e,t]       = pre_enc^T + att_conv^T                 (PSUM accumulation)
    th[e,t]      = tanh(z + dec[e] + b_enc[e])            (fused ACT bias)
    s[t]         = sum_e w_g[e] th[e,t]    (b_g dropped: softmax-invariant)
    em[t]        = mask01[t] exp(2 s[t])                  (unnormalized)
    a[t]         = em[t] / sum em                         (softmax)
    ctx[d]       = sum_t a[t] value[t,d]   (via V^T on DVE, fused mul+reduce)
"""

import numpy as np
import ml_dtypes

B, T, D = 16, 4096, 512
C, FW = 10, 100
KW = 2 * FW + 1           # 201
SHARP = 2.0
N_CORES = 8
BPC = B // N_CORES        # 2 batches per core
XW = T + 128              # im2col free width 4224
XPAD = T + 256            # padded att_prev length 4352

BF16 = ml_dtypes.bfloat16

_CACHE = {}


def _build_program():
    import concourse.bass as bass  # noqa: F401
    import concourse.tile as tile
    from concourse import bacc, mybir

    dt = mybir.dt
    f32 = dt.float32
    bf = dt.bfloat16
    AF = mybir.ActivationFunctionType
    ALU = mybir.AluOpType

    nc = bacc.Bacc(
        "TRN2",
        target_bir_lowering=False,
        debug=False,
        enable_asserts=False,
        num_devices=N_CORES,
    )

    # ---- I/O ----
    value_t = nc.dram_tensor("value_t", [BPC, D, T], bf, kind="ExternalInput").ap()
    x_im2col = nc.dram_tensor("x_im2col", [BPC, 128, XW], bf, kind="ExternalInput").ap()
    query_t = nc.dram_tensor("query_t", [D, BPC], bf, kind="ExternalInput").ap()
    mask_row = nc.dram_tensor("mask_row", [1, BPC * T], bf, kind="ExternalInput").ap()
    w_enc_t = nc.dram_tensor("w_enc_t", [128, D // 128 * D], bf, kind="ExternalInput").ap()
    w_dec_t = nc.dram_tensor("w_dec_t", [128, D // 128 * D], bf, kind="ExternalInput").ap()
    w_att_t = nc.dram_tensor("w_att_t", [C, D], bf, kind="ExternalInput").ap()
    w_conv = nc.dram_tensor("w_conv", [C, KW], bf, kind="ExternalInput").ap()
    wg = nc.dram_tensor("wg", [D, 1], bf, kind="ExternalInput").ap()
    becdup = nc.dram_tensor("becdup", [128, 8], f32, kind="ExternalInput").ap()
    ones_row = nc.dram_tensor("ones_row", [1, 128], f32, kind="ExternalInput").ap()
    ones_bf = nc.dram_tensor("ones_bf", [1, 128], bf, kind="ExternalInput").ap()
    id128 = nc.dram_tensor("id128", [128, 128], f32, kind="ExternalInput").ap()

    ctx_out = nc.dram_tensor("context", [BPC, D], f32, kind="ExternalOutput").ap()
    aw_out = nc.dram_tensor("att_weights", [BPC, T], f32, kind="ExternalOutput").ap()

    NG = T // 1024            # 4 z-groups of 1024
    NE = D // 128             # 4 e-chunks
    NK = D // 128             # 4 d-chunks
    NT = T // 512             # 8 score tiles

    with tile.TileContext(nc) as tc:
        with (
            tc.tile_pool(name="consts", bufs=1) as cpool,
            tc.tile_pool(name="vt", bufs=2) as vtpool,
            tc.tile_pool(name="xp", bufs=2) as xpool,
            tc.tile_pool(name="tanh", bufs=6) as thpool,
            tc.tile_pool(name="exf", bufs=3) as efpool,
            tc.tile_pool(name="exm", bufs=2) as empool,
            tc.tile_pool(name="bcs", bufs=3) as bcpool,
            tc.tile_pool(name="scr", bufs=2) as srpool,
            tc.tile_pool(name="small", bufs=2) as spool,
            tc.tile_pool(name="zps", bufs=2, space="PSUM") as zpool,
            tc.tile_pool(name="scps", bufs=2, space="PSUM") as scpool,
            tc.tile_pool(name="miscps", bufs=2, space="PSUM") as mpool,
        ):
            # ---- first-needed loads issued first, on the engines that start
            # fastest (Sync's HWDGE preamble delays it ~7us) ----
            vt_tiles = []
            for b in range(BPC):
                vt_tiles.append(vtpool.tile([128, NK, T], bf, tag="vt", name=f"vt{b}"))
            vt_srcs = [value_t[b].rearrange("(k p) t -> p k t", p=128) for b in range(BPC)]
            x_tiles = [xpool.tile([128, XW], bf, tag="x", name=f"x{b}") for b in range(BPC)]

            # tiny z-critical weights first (GpSimd starts issuing instantly)
            wc_sb = cpool.tile([C, KW], bf, tag="wc")
            nc.gpsimd.dma_start(wc_sb[:], w_conv)
            wa_sb = cpool.tile([C, D], bf, tag="wa")
            nc.gpsimd.dma_start(wa_sb[:], w_att_t)
            wg_sb = cpool.tile([128, NE], bf, tag="wg")
            nc.gpsimd.dma_start(wg_sb[:], wg.rearrange("(j p) o -> p (j o)", p=128))
            q_sb = cpool.tile([128, NK, BPC], bf, tag="q")
            nc.gpsimd.dma_start(q_sb[:], query_t.rearrange("(k p) b -> p k b", p=128))
            bec_sb = cpool.tile([128, 8], f32, tag="bec")
            nc.gpsimd.dma_start(bec_sb[:], becdup)
            we_sb = cpool.tile([128, NK, D], bf, tag="we")
            nc.scalar.dma_start(we_sb[:], w_enc_t.rearrange("p (k e) -> p k e", k=NK))
            wd_sb = cpool.tile([128, NK, D], bf, tag="wd")
            nc.scalar.dma_start(wd_sb[:], w_dec_t.rearrange("p (k e) -> p k e", k=NK))

            nc.gpsimd.dma_start(x_tiles[0][:], x_im2col[0])
            qeng = [nc.scalar, nc.gpsimd, nc.scalar, nc.gpsimd]
            for k in range(NK):
                qeng[k].dma_start(
                    vt_tiles[0][:, k, 0:1024], vt_srcs[0][:, k, 0:1024]
                )
            mrow_sb = cpool.tile([1, BPC * T], bf, tag="mrow")
            nc.sync.dma_start(mrow_sb[:], mask_row)
            onesr_sb = cpool.tile([1, 128], f32, tag="onesr")
            nc.sync.dma_start(onesr_sb[:], ones_row)
            onesb_sb = cpool.tile([1, 128], bf, tag="onesb")
            nc.sync.dma_start(onesb_sb[:], ones_bf)
            id128_sb = cpool.tile([128, 128], f32, tag="id128")
            nc.sync.dma_start(id128_sb[:], id128)

            # ---- G^T[k,e] = sum_c W_conv[c,k] W_att^T[c,e]  (conv ∘ att fold)
            g1_ps = zpool.tile([128, D], f32, tag="z")
            nc.tensor.matmul(g1_ps[:], wc_sb[:, 0:128], wa_sb[:])
            g1_sb = cpool.tile([128, D], bf, tag="g1")
            nc.vector.tensor_copy(g1_sb[:], g1_ps[:])
            g2_ps = zpool.tile([KW - 128, D], f32, tag="z")
            nc.tensor.matmul(g2_ps[:], wc_sb[:, 128:KW], wa_sb[:])
            g2_sb = cpool.tile([KW - 128, D], bf, tag="g2")
            nc.vector.tensor_copy(g2_sb[:], g2_ps[:])

            # ---- dec = W_dec @ query  -> bias[e, (j,b)] = dec + b_enc
            dec_ps = zpool.tile([128, NE * BPC], f32, tag="z")
            for j in range(NE):
                for k in range(NK):
                    nc.tensor.matmul(
                        dec_ps[:, BPC * j : BPC * (j + 1)],
                        wd_sb[:, k, 128 * j : 128 * (j + 1)],
                        q_sb[:, k, :],
                        start=(k == 0),
                        stop=(k == NK - 1),
                    )
            bias_sb = cpool.tile([128, NE * BPC], f32, tag="bias")
            nc.vector.tensor_add(bias_sb[:], dec_ps[:], bec_sb[:])

            for b in range(BPC):
                # ---- per-batch loads (quarter-major so compute starts early) ----
                vt_sb = vt_tiles[b]
                x_sb = x_tiles[b]
                if b > 0:
                    nc.gpsimd.dma_start(x_sb[:], x_im2col[b])
                engs3 = [nc.sync, nc.scalar, nc.gpsimd]
                for qt in range(4):
                    if b == 0 and qt == 0:
                        continue  # issued up-front
                    tsl = slice(1024 * qt, 1024 * (qt + 1))
                    for k in range(NK):
                        engs3[k % 3].dma_start(
                            vt_sb[:, k, tsl], vt_srcs[b][:, k, tsl]
                        )

                ctxparts = spool.tile([128, NK, NT], f32, tag="ctxparts")
                parts = spool.tile([1, NT], f32, tag="parts")
                em_row = empool.tile([1, T], bf, tag="em")

                # ---- z / tanh / scores / exp / bcast / ctx-accumulate ----
                for g in range(NG):
                    th_tiles = []
                    for j in range(NE):
                        z_ps = zpool.tile([128, 1024], f32, tag="z")
                        for h in range(2):
                            t0 = 1024 * g + 512 * h
                            sl = slice(512 * h, 512 * (h + 1))
                            for k in range(NK):
                                nc.tensor.matmul(
                                    z_ps[:, sl],
                                    we_sb[:, k, 128 * j : 128 * (j + 1)],
                                    vt_sb[:, k, t0 : t0 + 512],
                                    start=(k == 0),
                                    stop=False,
                                )
                            nc.tensor.matmul(
                                z_ps[:, sl],
                                g1_sb[:, 128 * j : 128 * (j + 1)],
                                x_sb[:, t0 : t0 + 512],
                                start=False,
                                stop=False,
                            )
                            nc.tensor.matmul(
                                z_ps[:, sl],
                                g2_sb[:, 128 * j : 128 * (j + 1)],
                                x_sb[0 : KW - 128, t0 + 128 : t0 + 640],
                                start=False,
                                stop=True,
                            )
                        th = thpool.tile([128, 1024], bf, tag="th")
                        nc.scalar.activation(
                            th[:], z_ps[:], AF.Tanh,
                            bias=bias_sb[:, BPC * j + b : BPC * j + b + 1],
                        )
                        th_tiles.append(th)
                    for h in range(2):
                        t0 = 1024 * g + 512 * h
                        it = 2 * g + h
                        sc_ps = scpool.tile([1, 512], f32, tag="sc")
                        for j in range(NE):
                            nc.tensor.matmul(
                                sc_ps[:],
                                wg_sb[:, j : j + 1],
                                th_tiles[j][:, 512 * h : 512 * (h + 1)],
                                start=(j == 0),
                                stop=(j == NE - 1),
                            )
                        ex = efpool.tile([1, 512], f32, tag="ex")
                        nc.scalar.activation(ex[:], sc_ps[:], AF.Exp, scale=SHARP)
                        # masked unnormalized weights (bf16) + partial sum
                        em = em_row[0:1, t0 : t0 + 512]
                        nc.vector.scalar_tensor_tensor(
                            em, ex[:], 1.0,
                            mrow_sb[0:1, b * T + t0 : b * T + t0 + 512],
                            ALU.mult, ALU.mult,
                            accum_out=parts[0:1, it : it + 1],
                        )
                        # broadcast em across partitions, ctx partial reduce
                        bc_ps = mpool.tile([128, 512], f32, tag="m")
                        nc.tensor.matmul(bc_ps[:], onesb_sb[:], em)
                        bc_sb = bcpool.tile([128, 512], bf, tag="bc")
                        nc.vector.tensor_copy(bc_sb[:], bc_ps[:])
                        for k in range(NK):
                            scr = srpool.tile([128, 1], bf, tag="scr")
                            nc.vector.scalar_tensor_tensor(
                                scr.broadcast_to((128, 512)),
                                vt_sb[:, k, t0 : t0 + 512],
                                1.0,
                                bc_sb[:],
                                ALU.mult, ALU.mult,
                                accum_out=ctxparts[:, k, it : it + 1],
                            )

                # ---- normalization ----
                ssum = spool.tile([1, 1], f32, tag="ssum")
                nc.vector.tensor_reduce(
                    ssum[:], parts[:], axis=mybir.AxisListType.X, op=ALU.add,
                )
                inv = spool.tile([1, 1], f32, tag="inv")
                nc.vector.reciprocal(inv[:], ssum[:])
                invb_ps = mpool.tile([128, 1], f32, tag="m")
                nc.tensor.matmul(invb_ps[:], onesr_sb[:], inv[:])
                invb_sb = spool.tile([128, 1], f32, tag="invb")
                nc.vector.tensor_copy(invb_sb[:], invb_ps[:])

                # context = (ctxcol * inv)^T
                ctxcol = spool.tile([128, NK], f32, tag="ctxcol")
                nc.vector.tensor_reduce(
                    ctxcol[:], ctxparts[:], axis=mybir.AxisListType.X, op=ALU.add,
                )
                ctxs = spool.tile([128, NK], f32, tag="ctxs")
                nc.vector.tensor_scalar_mul(ctxs[:], ctxcol[:], invb_sb[:])

                # att_weights = em * inv (row, fp32) — split across DVE and ACT
                aw_row = spool.tile([1, T], f32, tag="aw_row", bufs=1)
                nc.vector.tensor_scalar_mul(
                    aw_row[0:1, 0 : T // 2], em_row[0:1, 0 : T // 2], inv[:]
                )
                nc.scalar.mul(
                    aw_row[0:1, T // 2 : T], em_row[0:1, T // 2 : T], inv[:]
                )
                nc.gpsimd.dma_start(aw_out[b : b + 1, :], aw_row[:])
                ctxt_ps = mpool.tile([NK, 128], f32, tag="m")
                nc.tensor.transpose(ctxt_ps[:], ctxs[:], id128_sb[:])
                ctxt_sb = spool.tile([NK, 128], f32, tag="ctxt")
                nc.vector.tensor_copy(ctxt_sb[:], ctxt_ps[:])
                nc.gpsimd.dma_start(
                    ctx_out[b].rearrange("(k p) -> k p", p=128), ctxt_sb[:]
                )

    nc.compile()
    return nc


def _get_program():
    if "nc" not in _CACHE:
        _CACHE["nc"] = _build_program()
    return _CACHE["nc"]


def _host_prep(value, query, input_lengths, att_prev,
               W_enc, b_enc, W_dec, W_att, W_conv, w_g, b_g):
    """Build per-core input maps (layout prep only)."""
    value = np.asarray(value, np.float32)
    query = np.asarray(query, np.float32)
    lens = np.asarray(input_lengths).astype(np.int64)
    att_prev = np.asarray(att_prev, np.float32)

    vtb = np.ascontiguousarray(value.astype(BF16).transpose(0, 2, 1))  # [B,D,T]
    qtb = np.ascontiguousarray(query.T).astype(BF16)                   # [D,B]

    xpad = np.zeros((B, XPAD), np.float32)
    xpad[:, FW : FW + T] = att_prev
    xpad = xpad.astype(BF16)
    xw = np.lib.stride_tricks.sliding_window_view(xpad, XW, axis=1)
    xw = np.ascontiguousarray(xw[:, 0:128, :])                         # [B,128,XW]

    mask = (np.arange(T)[None, :] < lens[:, None]).astype(np.float32).astype(BF16)

    # packed [128, k*512]: row p holds W^T[128k+p, :] for k = 0..3
    w_enc_t = np.ascontiguousarray(
        np.asarray(W_enc, np.float32).T.reshape(4, 128, D).transpose(1, 0, 2).reshape(128, 4 * D)
    ).astype(BF16)
    w_dec_t = np.ascontiguousarray(
        np.asarray(W_dec, np.float32).T.reshape(4, 128, D).transpose(1, 0, 2).reshape(128, 4 * D)
    ).astype(BF16)
    w_att_t = np.ascontiguousarray(np.asarray(W_att, np.float32).T).astype(BF16)
    w_conv_n = np.ascontiguousarray(np.asarray(W_conv, np.float32)[:, 0, :]).astype(BF16)
    wg_col = np.ascontiguousarray(np.asarray(w_g, np.float32).reshape(D, 1)).astype(BF16)

    be = np.asarray(b_enc, np.float32)
    becdup = np.zeros((128, 8), np.float32)
    for j in range(4):
        for b in range(BPC):
            becdup[:, BPC * j + b] = be[128 * j : 128 * (j + 1)]

    ones_row = np.ones((1, 128), np.float32)
    ones_bf = np.ones((1, 128), np.float32).astype(BF16)
    id128 = np.eye(128, dtype=np.float32)

    in_maps = []
    for i in range(N_CORES):
        b0 = BPC * i
        sl = slice(b0, b0 + BPC)
        in_maps.append({
            "value_t": vtb[sl],
            "x_im2col": xw[sl],
            "query_t": np.ascontiguousarray(qtb[:, sl]),
            "mask_row": mask[sl].reshape(1, BPC * T),
            "w_enc_t": w_enc_t,
            "w_dec_t": w_dec_t,
            "w_att_t": w_att_t,
            "w_conv": w_conv_n,
            "wg": wg_col,
            "becdup": becdup,
            "ones_row": ones_row,
            "ones_bf": ones_bf,
            "id128": id128,
        })
    return in_maps


def run(trace=False, trace_kwargs=None, **inputs):
    from concourse.bass_utils import run_bass_kernel_spmd

    in_maps = _host_prep(**inputs)
    nc = _get_program()
    res = run_bass_kernel_spmd(
        nc, in_maps, core_ids=list(range(N_CORES)), trace=trace,
        **(trace_kwargs or {}),
    )
    ctx = np.concatenate([res.results[i]["context"] for i in range(N_CORES)], axis=0)
    aw = np.concatenate([res.results[i]["att_weights"] for i in range(N_CORES)], axis=0)
    return (ctx, aw), res


def kernel(**inputs):
    (ctx, aw), _ = run(trace=False, **inputs)
    return ctx, aw


if __name__ == "__main__":
    import reference

    inputs = {k: np.asarray(v) for k, v in reference.setup_inputs().items()}
    ctx, aw = kernel(**inputs)
    print("context", ctx.shape, "att_weights", aw.shape)
